# revision 7
# baseline (speedup 1.0000x reference)
"""AttnBlock (GroupNorm + spatial self-attention + residual) on 8 trn2 NeuronCores.

Sharding: 8 cores = 2 batches x 4 query-chunks of 1024 spatial positions.
Each core receives x[b] rolled so its query range is columns [0, 1024); all
cores run one identical SPMD program.

Host-side algebra (exact up to dropped softmax-invariant terms):
  scores^T[j,i] = hn[:,j] . (Wqk hn[:,i] + bqk)   with Wqk = C^-1/2 wk^T wq,
    bqk = C^-1/2 wk^T bq  (the bk term is constant over j -> softmax-invariant)
  out = x + sum_j softmax_j . (Wov hn[:,j]) + bov  with Wov = wo wv,
    bov = wo bv + bo      (softmax rows sum to 1 -> bias moves outside)

Device-side GroupNorm folding: hn = A.x + B per channel. The B-term of the
keys side is softmax-invariant; the A-scale folds into the projection weight
rows, and residual B-terms fold into runtime-adjusted biases. So the PE
reads raw x everywhere and no normalized copy is ever materialized.

fp8 DoubleRow: the four large matmul families (scores, hops, sums, vot) run
as fp8e4/e5 DoubleRow matmuls — two 128-deep k-planes per instruction with a
1024-wide fp8 moving stream — halving PE streaming time vs bf16. Scale
bookkeeping: wqk is host-scaled by 16 (qk8 lands in e4m3's sweet spot;
Exp(scale=1/16) undoes it), wov host-scaled by 128 (vot8 in e4m3;
onesr=1/128 folds the inverse into the softmax normalizer). exp(scores) is
stored e5m2 for dynamic range.

x arrives host-pre-blocked [NT, NS, 128, 512] so every DMA block is a
contiguous DRAM span (fewer descriptors -> faster issue).
"""

import ml_dtypes
import numpy as np

import concourse.bass as bass
import concourse.tile as tile
from concourse import bacc, mybir
from concourse import bass_utils

F32 = mybir.dt.float32
F32R = mybir.dt.float32r
BF16 = mybir.dt.bfloat16
F8E4 = mybir.dt.float8e4
F8E5 = mybir.dt.float8e5
DR = mybir.MatmulPerfMode.DoubleRow

B, C, D, H, W = 2, 512, 4, 32, 32
L = D * H * W            # 4096
G = 32                   # groupnorm groups
EPS = 1e-6
P = 128
NT = C // P              # 4 channel tiles
NJ = L // P              # 32 key tiles
NP = NJ // 2             # 16 key-tile pairs (DoubleRow granularity)
NS = L // 512            # 512-col blocks per channel chunk
IC = 512                 # i-chunk width
LQ = 1024                # query cols per core
NIC = LQ // IC           # 2 i-chunks
NCORES = 8
DEPTH = 3                # attention software-pipeline depth (es pair groups)

QK_SCALE = 16.0          # host-folded into wqk/wg/hqk; undone by Exp scale
OV_SCALE = 16.0          # host-folded into wov; undone in the output multiply
                         # (|vot| reaches ~8, so 16x keeps e4m3's 240 ceiling safe)

_CACHE = {}


def _build():
    nc = bacc.Bacc(trn_type="TRN2", target_bir_lowering=False, debug=False,
                   num_devices=NCORES)
    x_d = nc.dram_tensor("x", [NT, 2, P, L // 2], BF16, kind="ExternalInput").ap()
    xf_d = nc.dram_tensor("xf", [NT, P, LQ], F32R, kind="ExternalInput").ap()
    wqk_d = nc.dram_tensor("wqkT", [C, C], F32R, kind="ExternalInput").ap()
    wov_d = nc.dram_tensor("wovT", [C, C], BF16, kind="ExternalInput").ap()
    pg_d = nc.dram_tensor("pg", [C, G], F32R, kind="ExternalInput").ap()
    sel_d = nc.dram_tensor("sel", [G, C], F32R, kind="ExternalInput").ap()
    ones8_d = nc.dram_tensor("ones8", [P, 2 * P], F8E4, kind="ExternalInput").ap()
    gamma_d = nc.dram_tensor("gamma", [C], F32, kind="ExternalInput").ap()
    wg_d = nc.dram_tensor("wgT", [G, C], F32R, kind="ExternalInput").ap()
    vg_d = nc.dram_tensor("vgT", [G, C], F32R, kind="ExternalInput").ap()
    hqk_d = nc.dram_tensor("hqk", [C], F32, kind="ExternalInput").ap()
    hov_d = nc.dram_tensor("hov", [C], F32, kind="ExternalInput").ap()
    out_d = nc.dram_tensor("out", [C, LQ], F32, kind="ExternalOutput").ap()

    with tile.TileContext(nc) as tc:
        with (
            tc.tile_pool(name="big", bufs=1) as big,
            tc.tile_pool(name="wp", bufs=1) as wp,
            tc.tile_pool(name="small", bufs=1) as small,
            tc.tile_pool(name="est", bufs=DEPTH + 3) as est,
            tc.tile_pool(name="osb", bufs=4) as osb,
            tc.tile_pool(name="zp", bufs=6) as zp,
            tc.tile_pool(name="tmp", bufs=4) as tmp,
            tc.tile_pool(name="ps", bufs=3, space="PSUM") as ps,
            tc.tile_pool(name="pho", bufs=4, space="PSUM") as pho,
            tc.tile_pool(name="psum1", bufs=1, space="PSUM") as psum1,
        ):
            # ---- DMA strategy: one large contiguous DMA per tensor (a single
            # HWDGE instruction parallelizes internally at ~280GB/s; multiple
            # DMAs on one queue serialize). x chunks go on the sync queue so
            # they arrive pipelined for stats; everything else streams in
            # parallel on the gpsimd (SWDGE) queue. ----
            xt = big.tile([P, NT, L], BF16, tag="xt")
            for t in (0, 2, 1, 3):
                for h in range(2):
                    nc.sync.dma_start(xt[:, t, bass.ts(h, L // 2)], x_d[t, h])
            pg = small.tile([P, NT, G], F32R, tag="pg")
            nc.gpsimd.dma_start(pg[:], pg_d.rearrange("(t p) g -> p t g", p=P))
            sel = small.tile([G, NT, P], F32R, tag="sel")
            nc.gpsimd.dma_start(sel[:], sel_d.rearrange("g (t p) -> g t p", p=P))
            ones8 = small.tile([P, 2, P], F8E4, tag="ones8")
            nc.gpsimd.dma_start(ones8[:], ones8_d.rearrange("p (two q) -> p two q", two=2))
            gam = small.tile([P, NT], F32, tag="gam")
            nc.gpsimd.dma_start(gam[:], gamma_d.rearrange("(t p) -> p t", p=P))
            wg = small.tile([G, NT, P], F32R, tag="wg")
            nc.gpsimd.dma_start(wg[:], wg_d.rearrange("g (t p) -> g t p", p=P))
            vg = small.tile([G, NT, P], F32R, tag="vg")
            nc.gpsimd.dma_start(vg[:], vg_d.rearrange("g (t p) -> g t p", p=P))
            hqk = small.tile([P, NT], F32, tag="hqk")
            nc.gpsimd.dma_start(hqk[:], hqk_d.rearrange("(t p) -> p t", p=P))
            hov = small.tile([P, NT], F32, tag="hov")
            nc.gpsimd.dma_start(hov[:], hov_d.rearrange("(t p) -> p t", p=P))
            wqk = wp.tile([P, NT, C], F32R, tag="wqk")
            nc.sync.dma_start(wqk[:], wqk_d.rearrange("(t p) c -> p t c", p=P))
            xf = big.tile([P, NT, LQ], F32R, tag="xf")
            xf_r = xf_d.rearrange("t p l -> p t l")
            for icn in range(NIC):
                nc.sync.dma_start(xf[:, :, bass.ts(icn, IC)],
                                  xf_r[:, :, bass.ts(icn, IC)])
            wov = wp.tile([P, NT, C], BF16, tag="wov")
            nc.scalar.dma_start(wov[:], wov_d.rearrange("(t p) c -> p t c", p=P))

            # preload the Sqrt table set while DMA streams in
            epst = small.tile([G, 1], F32, tag="eps")
            nc.vector.memset(epst[:], EPS)
            dum = tmp.tile([G, 1], F32, tag="dum")
            nc.scalar.activation(dum[:], epst[:], mybir.ActivationFunctionType.Sqrt)

            # ---- fp8 copy of x for the DoubleRow matmul operand (GpSimd is
            # idle during the stream-in; conversion chases the DMA blocks) ----
            xt8 = big.tile([P, NT, L], F8E4, tag="xt8")
            for t in (0, 2, 1, 3):
                for h in range(2):
                    nc.gpsimd.tensor_scalar_mul(xt8[:, t, bass.ts(h, L // 2)],
                                                xt[:, t, bass.ts(h, L // 2)], 1.0)

            # ---- groupnorm stats: ACT accum_out for t=0 (lands first), DVE
            # bn_stats for t=1..3
            # (ACT Copy/Square passes with the free affine scale give mean and
            # E[x^2] directly; splitting engines shortens the serial chain) ----
            m2 = small.tile([P, NT, 2], F32R, tag="m2")
            ajunk = tmp.tile([P, L], BF16, tag="ajunk")
            for t in range(NT):
                if t in (0, 1):
                    macc = tmp.tile([P, 2], F32, tag="macc")
                    nc.scalar.activation(ajunk[:], xt[:, t, :],
                                         mybir.ActivationFunctionType.Copy,
                                         scale=1.0 / L, accum_out=macc[:, 0:1])
                    nc.scalar.activation(ajunk[:], xt[:, t, :],
                                         mybir.ActivationFunctionType.Square,
                                         scale=1.0 / (L ** 0.5),
                                         accum_out=macc[:, 1:2])
                    nc.vector.tensor_copy(m2[:, t, :], macc[:])
                    continue
                st = tmp.tile([P, NS, 6], F32, tag="bnst")
                for s in range(NS):
                    nc.vector.bn_stats(st[:, s, :], xt[:, t, bass.ts(s, 512)])
                mv = tmp.tile([P, 2], F32, tag="bnmv")
                nc.vector.bn_aggr(mv[:], st[:])
                # m2 = [mean, var + mean^2] = [mean, E[x^2]]
                msq = tmp.tile([P, 1], F32, tag="msq")
                nc.vector.tensor_mul(msq[:], mv[:, 0:1], mv[:, 0:1])
                nc.vector.tensor_copy(m2[:, t, 0:1], mv[:, 0:1])
                nc.vector.tensor_add(m2[:, t, 1:2], mv[:, 1:2], msq[:])
            gps = ps.tile([G, 2], F32, tag="mm")
            for t in range(NT):
                nc.tensor.matmul(gps[:], pg[:, t, :], m2[:, t, :],
                                 start=(t == 0), stop=(t == NT - 1))
            # group stats -> [mean_g, rstd_g]
            gsb = small.tile([G, 2], F32R, tag="gsb")
            nc.vector.tensor_copy(gsb[:, 0:1], gps[:, 0:1])
            vrg = tmp.tile([G, 1], F32, tag="vrg")
            nc.vector.tensor_mul(vrg[:], gsb[:, 0:1].bitcast(F32), gsb[:, 0:1].bitcast(F32))
            nc.vector.tensor_tensor(vrg[:], gps[:, 1:2], vrg[:], mybir.AluOpType.subtract)
            nc.scalar.activation(vrg[:], vrg[:], mybir.ActivationFunctionType.Sqrt,
                                 bias=epst[:], scale=1.0)
            with nc.allow_low_precision(reason="fp32r rounding of rstd is ~1e-4"):
                nc.vector.reciprocal(gsb[:, 1:2], vrg[:])
            # broadcast to channels: chsb[p, t, 0:2] = [mean, rstd] per channel
            # (4 single-MM groups share one PSUM tile; finished groups only
            # lose has_written bits, not data)
            chsb = small.tile([P, NT, 2], F32, tag="chsb")
            chs = ps.tile([P, 2 * NT], F32, tag="mm")
            for t in range(NT):
                nc.tensor.matmul(chs[:, 2 * t:2 * t + 2], sel[:, t, :], gsb[:],
                                 start=True, stop=True)
            nc.vector.tensor_copy(chsb[:], chs[:])
            # A = rstd*gamma per channel
            A = small.tile([P, NT], F32, tag="A")
            nc.vector.tensor_mul(A[:], chsb[:, :, 1], gam[:])

            # ---- scale weight rows by A in place (TT w/ free-dim-0 broadcast:
            # tensor_scalar with AP scalars has a ~3.3us ucode overhead) ----
            for tq in range(NT):
                nc.vector.tensor_tensor(wqk[:, :, bass.ts(tq, P)],
                                        wqk[:, :, bass.ts(tq, P)],
                                        A[:, :, None].to_broadcast((P, NT, P)),
                                        mybir.AluOpType.mult)
            # wov8 = (host-prescaled wov*128) * A, quantized e4m3
            wov8 = wp.tile([P, NT, C], F8E4, tag="wov8")
            for t in range(NT):
                eng = nc.vector if t < 2 else nc.gpsimd
                eng.tensor_tensor(wov8[:, t, :], wov[:, t, :],
                                  A[:, t:t + 1].to_broadcast((P, C)),
                                  mybir.AluOpType.mult)

            # ---- qk8[:, i] = A . (Wqk16A x_sl + bqkE16) over query cols,
            # written e4m3 (the host 16x makes the fp8 values ~unit-scale) ----
            qk8 = big.tile([P, NT, LQ], F8E4, tag="qk8")
            emitted_bias = [False]

            bias_tiles = {}

            def emit_bias():
                # s_g = mean_g * rstd_g (padded to 2 cols for fp32r matmul rhs)
                st2 = small.tile([G, 2], F32R, tag="st2")
                nc.vector.tensor_mul(st2[:, 0:1], gsb[:, 0:1].bitcast(F32), gsb[:, 1:2].bitcast(F32))
                nc.vector.tensor_copy(st2[:, 1:2], gsb[:, 0:1].bitcast(F32))
                # bias folds: bqkE = hqk - Wg.s, bovE = hov - Vg.s (host-folded
                # per-group partial matvecs; 8 tiny K=32 MMs off the critical chain)
                bqkE = small.tile([P, NT], F32, tag="bqkE")
                bovE = small.tile([P, NT], F32, tag="bovE")
                psB = ps.tile([P, 4 * NT], F32, tag="mm")
                for tq in range(NT):
                    nc.tensor.matmul(psB[:, 2 * tq:2 * tq + 2], wg[:, tq, :], st2[:],
                                         start=True, stop=True)
                    nc.tensor.matmul(psB[:, 2 * NT + 2 * tq:2 * NT + 2 * tq + 2],
                                         vg[:, tq, :], st2[:], start=True, stop=True)
                psBv = psB.rearrange("p (c two) -> p c two", two=2)
                nc.vector.tensor_tensor(bqkE[:], hqk[:], psBv[:, 0:NT, 0],
                                                mybir.AluOpType.subtract)
                nc.vector.tensor_tensor(bovE[:], hov[:], psBv[:, NT:2 * NT, 0],
                                                mybir.AluOpType.subtract)
                bias_tiles["bqkE"] = bqkE
                bias_tiles["bovE"] = bovE
                emitted_bias[0] = True

            for icn in range(NIC):
                for tq in range(NT):
                    qps = ps.tile([P, IC], F32, tag="mm")
                    for t in range(NT):
                        nc.tensor.matmul(qps[:], wqk[:, t, bass.ts(tq, P)],
                                         xf[:, t, bass.ts(icn, IC)],
                                         start=(t == 0), stop=(t == NT - 1))
                    if not emitted_bias[0]:
                        emit_bias()
                    # qk8 = (qps + bqkE16) * A, one fused DVE op, e4m3 out
                    nc.vector.scalar_tensor_tensor(
                        qk8[:, tq, bass.ts(icn, IC)], qps[:],
                        bias_tiles["bqkE"][:, tq:tq + 1],
                        A[:, tq:tq + 1].to_broadcast((P, IC)),
                        mybir.AluOpType.add, mybir.AluOpType.mult)

            if not emitted_bias[0]:
                emit_bias()

            # ---- z[t][icn] = x_residual + bovE on GpSimd (idle here) ----
            zall = {}
            for icn in range(NIC):
                for t in range(NT):
                    z = zp.tile([P, IC], F32, tag="zp", name=f"z{icn}_{t}")
                    nc.gpsimd.tensor_tensor(z[:], xf[:, t, bass.ts(icn, IC)].bitcast(F32),
                                            bias_tiles["bovE"][:, t:t + 1].to_broadcast((P, IC)),
                                            mybir.AluOpType.add)
                    zall[(icn, t)] = z

            # ---- voT projection: vot8[j, c] = 128 * (WovA x)[c, j]^T, e4m3.
            # DoubleRow: 2 channel-tile k-planes per MM. ----
            vot8 = big.tile([P, NJ, C], F8E4, tag="vot8")
            for j in range(NJ):
                vps = ps.tile([P, C], F32, tag="mm")
                for u in range(2):
                    nc.tensor.matmul(vps[:], xt8[:, 2 * u:2 * u + 2, bass.ts(j, P)],
                                     wov8[:, 2 * u:2 * u + 2, :],
                                     start=(u == 0), stop=(u == 1), perf_mode=DR)
                nc.vector.tensor_copy(vot8[:, j, :], vps[:])

            # ---- attention per i-chunk (DoubleRow over key-tile pairs) ----
            pending_fin = [None]

            def make_finalize(icn, sums, hops):
                def fin():
                    # every row of sums is the same column-sum (all-ones lhsT),
                    # so the reciprocal IS the broadcast — no PE round-trip.
                    zs = [zall[(icn, t)] for t in range(NT)]
                    last = icn == NIC - 1
                    nh = 2 if last else 1
                    HW_ = IC // nh
                    for h in range(nh):
                        sl = slice(h * HW_, (h + 1) * HW_)
                        rbc = tmp.tile([P, HW_], F32, tag="rbc", name=f"rbc{icn}_{h}")
                        nc.vector.reciprocal_approx_fast(rbc[:], sums[:, sl])
                        for t in range(NT):
                            o = osb.tile([P, HW_], F32, tag="osb",
                                         name=f"o{icn}_{h}_{t}")
                            # o = (hops / OV_SCALE) * (1/sums), one fused op
                            # (PSUM read -> DVE only; the z add can go to GpSimd)
                            nc.vector.scalar_tensor_tensor(o[:], hops[t][:, sl],
                                                           1.0 / OV_SCALE, rbc[:],
                                                           mybir.AluOpType.mult,
                                                           mybir.AluOpType.mult)
                            eng = nc.gpsimd if (last and t % 2) else nc.vector
                            eng.tensor_tensor(o[:], o[:], zs[t][:, sl],
                                              mybir.AluOpType.add)
                            nc.sync.dma_start(
                                out_d[bass.ts(t, P),
                                      icn * IC + h * HW_:icn * IC + (h + 1) * HW_],
                                o[:])
                return fin

            for icn in range(NIC):
                sums = psum1.tile([P, IC], F32, tag="sums", name=f"sums{icn}")
                hops = [pho.tile([P, IC], F32, tag="ho", name=f"ho_{icn}_{t}")
                        for t in range(NT)]
                ests = [None] * NP

                def consume(w, sums=sums, hops=hops, ests=ests):
                    es2 = ests[w]
                    nc.tensor.matmul(sums[:], ones8[:], es2[:],
                                     start=(w == 0), stop=(w == NP - 1),
                                     perf_mode=DR)
                    for t in range(NT):
                        nc.tensor.matmul(hops[t][:],
                                         vot8[:, 2 * w:2 * w + 2, bass.ts(t, P)],
                                         es2[:],
                                         start=(w == 0), stop=(w == NP - 1),
                                         perf_mode=DR)
                    ests[w] = None

                for w in range(NP):
                    if w == 1 and pending_fin[0] is not None:
                        pending_fin[0]()
                        pending_fin[0] = None
                    es2 = est.tile([P, 2, IC], F8E5, tag="est",
                                   name=f"est{icn}_{w}")
                    for hh in range(2):
                        j = 2 * w + hh
                        sps = ps.tile([P, IC], F32, tag="mm", name=f"sps{icn}_{j}")
                        for u in range(2):
                            nc.tensor.matmul(sps[:],
                                             xt8[:, 2 * u:2 * u + 2, bass.ts(j, P)],
                                             qk8[:, 2 * u:2 * u + 2, bass.ts(icn, IC)],
                                             start=(u == 0), stop=(u == 1),
                                             perf_mode=DR)
                        nc.scalar.activation(es2[:, hh, :], sps[:],
                                             mybir.ActivationFunctionType.Exp,
                                             scale=1.0 / QK_SCALE)
                    ests[w] = es2
                    if w >= DEPTH:
                        consume(w - DEPTH)
                for w in range(NP - DEPTH, NP):
                    consume(w)
                pending_fin[0] = make_finalize(icn, sums, hops)
            pending_fin[0]()

    nc.compile()
    return nc


def _prep(inputs):
    s = float(C) ** -0.5
    wq = np.asarray(inputs["wq"], np.float64)
    wk = np.asarray(inputs["wk"], np.float64)
    wv = np.asarray(inputs["wv"], np.float64)
    wo = np.asarray(inputs["wo"], np.float64)
    bq = np.asarray(inputs["bq"], np.float64)
    bv = np.asarray(inputs["bv"], np.float64)
    bo = np.asarray(inputs["bo"], np.float64)
    gamma = np.asarray(inputs["gamma"], np.float64)
    beta = np.asarray(inputs["beta"], np.float64)
    Wqk = (wk.T @ wq).T * s      # lhsT layout [c_in, c_out]; Wqk.T is [c_out, c_in]
    Wov = (wo @ wv).T            # [c_in, c_out]
    bqkv = (wk.T @ bq) * s
    bovv = wo @ bv + bo
    # per-group partial matvecs with gamma folded: WgT[g, c] = sum_{c' in g} Wqk[c,c'] gamma[c']
    GS = C // G
    WgT = (Wqk * gamma[:, None]).reshape(G, GS, C).sum(axis=1)
    VgT = (Wov * gamma[:, None]).reshape(G, GS, C).sum(axis=1)
    consts = {
        "wqkT": np.ascontiguousarray(Wqk * QK_SCALE, np.float32),
        "wovT": np.ascontiguousarray(Wov * OV_SCALE).astype(ml_dtypes.bfloat16),
        "wgT": np.ascontiguousarray(WgT * QK_SCALE, np.float32),
        "vgT": np.ascontiguousarray(VgT, np.float32),
        "hqk": ((Wqk.T @ beta + bqkv) * QK_SCALE).astype(np.float32),
        "hov": (Wov.T @ beta + bovv).astype(np.float32),
        "gamma": np.asarray(inputs["gamma"], np.float32),
        "pg": np.ascontiguousarray(
            (np.arange(C)[:, None] // (C // G) == np.arange(G)[None, :])
            .astype(np.float32) / (C // G)),
        "sel": np.ascontiguousarray(
            (np.arange(G)[:, None] == np.arange(C)[None, :] // (C // G))
            .astype(np.float32)),
        "ones8": np.ones((P, 2 * P), ml_dtypes.float8_e4m3),
    }
    return consts


LAST_RESULTS = None


def kernel(**inputs) -> np.ndarray:
    global LAST_RESULTS
    if "nc" not in _CACHE:
        _CACHE["nc"] = _build()
    nc = _CACHE["nc"]
    consts = _prep(inputs)
    x = np.asarray(inputs["x"], np.float32)
    xb = x.reshape(B, C, L)
    in_maps = []
    for core in range(NCORES):
        b, chunk = divmod(core, 4)
        xr = np.roll(xb[b], -LQ * chunk, axis=1)
        xblk = np.ascontiguousarray(
            xr.reshape(NT, P, 2, L // 2).swapaxes(1, 2)).astype(ml_dtypes.bfloat16)
        xf = np.ascontiguousarray(xr[:, :LQ].reshape(NT, P, LQ))
        in_maps.append({"x": xblk, "xf": xf, **consts})
    res = bass_utils.run_bass_kernel_spmd(nc, in_maps, core_ids=list(range(NCORES)))
    LAST_RESULTS = res
    out = np.empty((B, C, L), np.float32)
    for core in range(NCORES):
        b, chunk = divmod(core, 4)
        out[b][:, LQ * chunk:LQ * (chunk + 1)] = res.results[core]["out"]
    return out.reshape(B, C, D, H, W)


# revision 12
# speedup vs baseline: 2.5020x; 2.5020x over previous
"""AttnBlock (GroupNorm + spatial self-attention + residual) on 8 trn2 NeuronCores.

Sharding: 8 cores = 2 batches x 4 query-chunks of 1024 spatial positions.
Each core receives x[b] rolled so its query range is columns [0, 1024); all
cores run one identical SPMD program.

Host-side algebra (exact up to dropped softmax-invariant terms):
  scores^T[j,i] = hn[:,j] . (Wqk hn[:,i] + bqk)   with Wqk = C^-1/2 wk^T wq,
    bqk = C^-1/2 wk^T bq  (the bk term is constant over j -> softmax-invariant)
  out = x + sum_j softmax_j . (Wov hn[:,j]) + bov  with Wov = wo wv,
    bov = wo bv + bo      (softmax rows sum to 1 -> bias moves outside)

Device-side GroupNorm folding: hn = A.x + B per channel. The B-term of the
keys side is softmax-invariant; the A-scale folds into the projection weight
rows, and residual B-terms fold into runtime-adjusted biases. So the PE
reads raw x everywhere and no normalized copy is ever materialized.

fp8 DoubleRow: the four large matmul families (scores, hops, sums, vot) run
as fp8e4/e5 DoubleRow matmuls — two 128-deep k-planes per instruction with a
1024-wide fp8 moving stream — halving PE streaming time vs bf16. Scale
bookkeeping: wqk is host-scaled by 16 (qk8 lands in e4m3's sweet spot;
Exp(scale=1/16) undoes it), wov host-scaled by 128 (vot8 in e4m3;
onesr=1/128 folds the inverse into the softmax normalizer). exp(scores) is
stored e5m2 for dynamic range.

x arrives host-pre-blocked [NT, NS, 128, 512] so every DMA block is a
contiguous DRAM span (fewer descriptors -> faster issue).
"""

import ml_dtypes
import numpy as np

import concourse.bass as bass
import concourse.tile as tile
from concourse import bacc, mybir
from concourse import bass_utils

F32 = mybir.dt.float32
F32R = mybir.dt.float32r
BF16 = mybir.dt.bfloat16
F8E4 = mybir.dt.float8e4
F8E5 = mybir.dt.float8e5
DR = mybir.MatmulPerfMode.DoubleRow

B, C, D, H, W = 2, 512, 4, 32, 32
L = D * H * W            # 4096
G = 32                   # groupnorm groups
EPS = 1e-6
P = 128
NT = C // P              # 4 channel tiles
NJ = L // P              # 32 key tiles
NP = NJ // 2             # 16 key-tile pairs (DoubleRow granularity)
NS = L // 512            # 512-col blocks per channel chunk
IC = 512                 # i-chunk width
LQ = 1024                # query cols per core
NIC = LQ // IC           # 2 i-chunks
NCORES = 8
DEPTH = 3                # attention software-pipeline depth (es pair groups)

QK_SCALE = 16.0          # host-folded into wqk/wg/hqk; undone by Exp scale
OV_SCALE = 16.0          # host-folded into wov; undone in the output multiply
                         # (|vot| reaches ~8, so 16x keeps e4m3's 240 ceiling safe)

_CACHE = {}


def _build():
    nc = bacc.Bacc(trn_type="TRN2", target_bir_lowering=False, debug=False,
                   num_devices=NCORES)
    x_d = nc.dram_tensor("x", [NT, 2, P, L // 2], BF16, kind="ExternalInput").ap()
    x8_d = nc.dram_tensor("x8", [NT, 2, P, L // 2], F8E4, kind="ExternalInput").ap()
    xf_d = nc.dram_tensor("xf", [NT, P, LQ], F32R, kind="ExternalInput").ap()
    wqk_d = nc.dram_tensor("wqkT", [C, C], F32R, kind="ExternalInput").ap()
    wov_d = nc.dram_tensor("wovT", [C, C], BF16, kind="ExternalInput").ap()
    pg_d = nc.dram_tensor("pg", [C, G], F32R, kind="ExternalInput").ap()
    sel_d = nc.dram_tensor("sel", [G, C], F32R, kind="ExternalInput").ap()
    ones8_d = nc.dram_tensor("ones8", [P, 2 * P], F8E4, kind="ExternalInput").ap()
    gamma_d = nc.dram_tensor("gamma", [C], F32, kind="ExternalInput").ap()
    wg_d = nc.dram_tensor("wgT", [G, C], F32R, kind="ExternalInput").ap()
    vg_d = nc.dram_tensor("vgT", [G, C], F32R, kind="ExternalInput").ap()
    hqk_d = nc.dram_tensor("hqk", [C], F32, kind="ExternalInput").ap()
    hov_d = nc.dram_tensor("hov", [C], F32, kind="ExternalInput").ap()
    out_d = nc.dram_tensor("out", [C, LQ], F32, kind="ExternalOutput").ap()

    with tile.TileContext(nc) as tc:
        with (
            tc.tile_pool(name="big", bufs=1) as big,
            tc.tile_pool(name="wp", bufs=1) as wp,
            tc.tile_pool(name="small", bufs=1) as small,
            tc.tile_pool(name="est", bufs=DEPTH + 3) as est,
            tc.tile_pool(name="osb", bufs=4) as osb,
            tc.tile_pool(name="zp", bufs=6) as zp,
            tc.tile_pool(name="tmp", bufs=4) as tmp,
            tc.tile_pool(name="ps", bufs=3, space="PSUM") as ps,
            tc.tile_pool(name="pho", bufs=4, space="PSUM") as pho,
            tc.tile_pool(name="psum1", bufs=1, space="PSUM") as psum1,
        ):
            # ---- DMA strategy: one large contiguous DMA per tensor (a single
            # HWDGE instruction parallelizes internally at ~280GB/s; multiple
            # DMAs on one queue serialize). x chunks go on the sync queue so
            # they arrive pipelined for stats; everything else streams in
            # parallel on the gpsimd (SWDGE) queue. ----
            xt = big.tile([P, NT, L], BF16, tag="xt")
            for t in (0, 2, 1, 3):
                for h in range(2):
                    nc.sync.dma_start(xt[:, t, bass.ts(h, L // 2)], x_d[t, h])
            pg = small.tile([P, NT, G], F32R, tag="pg")
            nc.gpsimd.dma_start(pg[:], pg_d.rearrange("(t p) g -> p t g", p=P))
            sel = small.tile([G, NT, P], F32R, tag="sel")
            nc.gpsimd.dma_start(sel[:], sel_d.rearrange("g (t p) -> g t p", p=P))
            ones8 = small.tile([P, 2, P], F8E4, tag="ones8")
            nc.gpsimd.dma_start(ones8[:], ones8_d.rearrange("p (two q) -> p two q", two=2))
            gam = small.tile([P, NT], F32, tag="gam")
            nc.gpsimd.dma_start(gam[:], gamma_d.rearrange("(t p) -> p t", p=P))
            wg = small.tile([G, NT, P], F32R, tag="wg")
            nc.gpsimd.dma_start(wg[:], wg_d.rearrange("g (t p) -> g t p", p=P))
            vg = small.tile([G, NT, P], F32R, tag="vg")
            nc.gpsimd.dma_start(vg[:], vg_d.rearrange("g (t p) -> g t p", p=P))
            hqk = small.tile([P, NT], F32, tag="hqk")
            nc.gpsimd.dma_start(hqk[:], hqk_d.rearrange("(t p) -> p t", p=P))
            hov = small.tile([P, NT], F32, tag="hov")
            nc.gpsimd.dma_start(hov[:], hov_d.rearrange("(t p) -> p t", p=P))
            wqk = wp.tile([P, NT, C], F32R, tag="wqk")
            nc.sync.dma_start(wqk[:], wqk_d.rearrange("(t p) c -> p t c", p=P))
            xf = big.tile([P, NT, LQ], F32R, tag="xf")
            xf_r = xf_d.rearrange("t p l -> p t l")
            for icn in range(NIC):
                nc.sync.dma_start(xf[:, :, bass.ts(icn, IC)],
                                  xf_r[:, :, bass.ts(icn, IC)])
            wov = wp.tile([P, NT, C], BF16, tag="wov")
            nc.scalar.dma_start(wov[:], wov_d.rearrange("(t p) c -> p t c", p=P))
            # fp8 copy of x, host-converted (on-device fp8 conversion is only
            # fast on the ACT engine; GpSimd/DVE ucode paths are 15-40x slow)
            xt8 = big.tile([P, NT, L], F8E4, tag="xt8")
            for t in range(NT):
                for h in range(2):
                    nc.scalar.dma_start(xt8[:, t, bass.ts(h, L // 2)], x8_d[t, h])

            # preload the Sqrt table set while DMA streams in
            epst = small.tile([G, 1], F32, tag="eps")
            nc.vector.memset(epst[:], EPS)
            dum = tmp.tile([G, 1], F32, tag="dum")
            nc.scalar.activation(dum[:], epst[:], mybir.ActivationFunctionType.Sqrt)

            # ---- groupnorm stats: ACT accum_out for t=0 (lands first), DVE
            # bn_stats for t=1..3
            # (ACT Copy/Square passes with the free affine scale give mean and
            # E[x^2] directly; splitting engines shortens the serial chain) ----
            m2 = small.tile([P, NT, 2], F32R, tag="m2")
            ajunk = tmp.tile([P, L], BF16, tag="ajunk")
            for t in range(NT):
                if t in (0, 1):
                    macc = tmp.tile([P, 2], F32, tag="macc")
                    nc.scalar.activation(ajunk[:], xt[:, t, :],
                                         mybir.ActivationFunctionType.Copy,
                                         scale=1.0 / L, accum_out=macc[:, 0:1])
                    nc.scalar.activation(ajunk[:], xt[:, t, :],
                                         mybir.ActivationFunctionType.Square,
                                         scale=1.0 / (L ** 0.5),
                                         accum_out=macc[:, 1:2])
                    nc.vector.tensor_copy(m2[:, t, :], macc[:])
                    continue
                st = tmp.tile([P, NS, 6], F32, tag="bnst")
                for s in range(NS):
                    nc.vector.bn_stats(st[:, s, :], xt[:, t, bass.ts(s, 512)])
                mv = tmp.tile([P, 2], F32, tag="bnmv")
                nc.vector.bn_aggr(mv[:], st[:])
                # m2 = [mean, var + mean^2] = [mean, E[x^2]]
                msq = tmp.tile([P, 1], F32, tag="msq")
                nc.vector.tensor_mul(msq[:], mv[:, 0:1], mv[:, 0:1])
                nc.vector.tensor_copy(m2[:, t, 0:1], mv[:, 0:1])
                nc.vector.tensor_add(m2[:, t, 1:2], mv[:, 1:2], msq[:])
            gps = ps.tile([G, 2], F32, tag="mm")
            for t in range(NT):
                nc.tensor.matmul(gps[:], pg[:, t, :], m2[:, t, :],
                                 start=(t == 0), stop=(t == NT - 1))
            # group stats -> [mean_g, rstd_g]
            gsb = small.tile([G, 2], F32R, tag="gsb")
            nc.vector.tensor_copy(gsb[:, 0:1], gps[:, 0:1])
            vrg = tmp.tile([G, 1], F32, tag="vrg")
            nc.vector.tensor_mul(vrg[:], gsb[:, 0:1].bitcast(F32), gsb[:, 0:1].bitcast(F32))
            nc.vector.tensor_tensor(vrg[:], gps[:, 1:2], vrg[:], mybir.AluOpType.subtract)
            nc.scalar.activation(vrg[:], vrg[:], mybir.ActivationFunctionType.Sqrt,
                                 bias=epst[:], scale=1.0)
            with nc.allow_low_precision(reason="fp32r rounding of rstd is ~1e-4"):
                nc.vector.reciprocal(gsb[:, 1:2], vrg[:])
            # broadcast to channels: chsb[p, t, 0:2] = [mean, rstd] per channel
            # (4 single-MM groups share one PSUM tile; finished groups only
            # lose has_written bits, not data)
            chsb = small.tile([P, NT, 2], F32, tag="chsb")
            chs = ps.tile([P, 2 * NT], F32, tag="mm")
            for t in range(NT):
                nc.tensor.matmul(chs[:, 2 * t:2 * t + 2], sel[:, t, :], gsb[:],
                                 start=True, stop=True)
            nc.vector.tensor_copy(chsb[:], chs[:])
            # A = rstd*gamma per channel
            A = small.tile([P, NT], F32, tag="A")
            nc.vector.tensor_mul(A[:], chsb[:, :, 1], gam[:])

            # ---- scale weight rows by A in place (TT w/ free-dim-0 broadcast:
            # tensor_scalar with AP scalars has a ~3.3us ucode overhead) ----
            for tq in range(NT):
                nc.vector.tensor_tensor(wqk[:, :, bass.ts(tq, P)],
                                        wqk[:, :, bass.ts(tq, P)],
                                        A[:, :, None].to_broadcast((P, NT, P)),
                                        mybir.AluOpType.mult)
            # wov8 = (host-prescaled wov*16) * A, quantized e4m3 (DVE fp8-out
            # ALU ops are fine; GpSimd's are not — keep all four on Vector)
            wov8 = wp.tile([P, NT, C], F8E4, tag="wov8")
            for t in range(NT):
                nc.vector.tensor_tensor(wov8[:, t, :], wov[:, t, :],
                                        A[:, t:t + 1].to_broadcast((P, C)),
                                        mybir.AluOpType.mult)

            # ---- qk8[:, i] = A . (Wqk16A x_sl + bqkE16) over query cols,
            # written e4m3 (the host 16x makes the fp8 values ~unit-scale) ----
            qk8 = big.tile([P, NT, LQ], F8E4, tag="qk8")
            emitted_bias = [False]

            bias_tiles = {}

            def emit_bias():
                # s_g = mean_g * rstd_g (padded to 2 cols for fp32r matmul rhs)
                st2 = small.tile([G, 2], F32R, tag="st2")
                nc.vector.tensor_mul(st2[:, 0:1], gsb[:, 0:1].bitcast(F32), gsb[:, 1:2].bitcast(F32))
                nc.vector.tensor_copy(st2[:, 1:2], gsb[:, 0:1].bitcast(F32))
                # bias folds: bqkE = hqk - Wg.s, bovE = hov - Vg.s (host-folded
                # per-group partial matvecs; 8 tiny K=32 MMs off the critical chain)
                bqkE = small.tile([P, NT], F32, tag="bqkE")
                bovE = small.tile([P, NT], F32, tag="bovE")
                psB = ps.tile([P, 4 * NT], F32, tag="mm")
                for tq in range(NT):
                    nc.tensor.matmul(psB[:, 2 * tq:2 * tq + 2], wg[:, tq, :], st2[:],
                                         start=True, stop=True)
                    nc.tensor.matmul(psB[:, 2 * NT + 2 * tq:2 * NT + 2 * tq + 2],
                                         vg[:, tq, :], st2[:], start=True, stop=True)
                psBv = psB.rearrange("p (c two) -> p c two", two=2)
                nc.vector.tensor_tensor(bqkE[:], hqk[:], psBv[:, 0:NT, 0],
                                                mybir.AluOpType.subtract)
                nc.vector.tensor_tensor(bovE[:], hov[:], psBv[:, NT:2 * NT, 0],
                                                mybir.AluOpType.subtract)
                bias_tiles["bqkE"] = bqkE
                bias_tiles["bovE"] = bovE
                emitted_bias[0] = True

            for icn in range(NIC):
                for tq in range(NT):
                    qps = ps.tile([P, IC], F32, tag="mm")
                    for t in range(NT):
                        nc.tensor.matmul(qps[:], wqk[:, t, bass.ts(tq, P)],
                                         xf[:, t, bass.ts(icn, IC)],
                                         start=(t == 0), stop=(t == NT - 1))
                    if not emitted_bias[0]:
                        emit_bias()
                    # qk8 = (qps + bqkE16) * A, one fused DVE op, e4m3 out
                    nc.vector.scalar_tensor_tensor(
                        qk8[:, tq, bass.ts(icn, IC)], qps[:],
                        bias_tiles["bqkE"][:, tq:tq + 1],
                        A[:, tq:tq + 1].to_broadcast((P, IC)),
                        mybir.AluOpType.add, mybir.AluOpType.mult)

            if not emitted_bias[0]:
                emit_bias()

            # ---- z[t][icn] = x_residual + bovE on GpSimd (idle here) ----
            zall = {}
            for icn in range(NIC):
                for t in range(NT):
                    z = zp.tile([P, IC], F32, tag="zp", name=f"z{icn}_{t}")
                    nc.gpsimd.tensor_tensor(z[:], xf[:, t, bass.ts(icn, IC)].bitcast(F32),
                                            bias_tiles["bovE"][:, t:t + 1].to_broadcast((P, IC)),
                                            mybir.AluOpType.add)
                    zall[(icn, t)] = z

            # ---- voT projection: vot8[j, c] = 128 * (WovA x)[c, j]^T, e4m3.
            # DoubleRow: 2 channel-tile k-planes per MM. ----
            vot8 = big.tile([P, NJ, C], F8E4, tag="vot8")
            for j in range(NJ):
                vps = ps.tile([P, C], F32, tag="mm")
                for u in range(2):
                    nc.tensor.matmul(vps[:], xt8[:, 2 * u:2 * u + 2, bass.ts(j, P)],
                                     wov8[:, 2 * u:2 * u + 2, :],
                                     start=(u == 0), stop=(u == 1), perf_mode=DR)
                nc.vector.tensor_copy(vot8[:, j, :], vps[:])

            # ---- attention per i-chunk (DoubleRow over key-tile pairs) ----
            pending_fin = [None]

            def make_finalize(icn, sums, hops):
                def fin():
                    # every row of sums is the same column-sum (all-ones lhsT),
                    # so the reciprocal IS the broadcast — no PE round-trip.
                    zs = [zall[(icn, t)] for t in range(NT)]
                    last = icn == NIC - 1
                    nh = 2 if last else 1
                    HW_ = IC // nh
                    for h in range(nh):
                        sl = slice(h * HW_, (h + 1) * HW_)
                        rbc = tmp.tile([P, HW_], F32, tag="rbc", name=f"rbc{icn}_{h}")
                        nc.vector.reciprocal_approx_fast(rbc[:], sums[:, sl])
                        for t in range(NT):
                            o = osb.tile([P, HW_], F32, tag="osb",
                                         name=f"o{icn}_{h}_{t}")
                            # o = (hops / OV_SCALE) * (1/sums), one fused op
                            # (PSUM read -> DVE only; the z add can go to GpSimd)
                            nc.vector.scalar_tensor_tensor(o[:], hops[t][:, sl],
                                                           1.0 / OV_SCALE, rbc[:],
                                                           mybir.AluOpType.mult,
                                                           mybir.AluOpType.mult)
                            eng = nc.gpsimd if (last and t % 2) else nc.vector
                            eng.tensor_tensor(o[:], o[:], zs[t][:, sl],
                                              mybir.AluOpType.add)
                            nc.sync.dma_start(
                                out_d[bass.ts(t, P),
                                      icn * IC + h * HW_:icn * IC + (h + 1) * HW_],
                                o[:])
                return fin

            for icn in range(NIC):
                sums = psum1.tile([P, IC], F32, tag="sums", name=f"sums{icn}")
                hops = [pho.tile([P, IC], F32, tag="ho", name=f"ho_{icn}_{t}")
                        for t in range(NT)]
                ests = [None] * NP

                def consume(w, sums=sums, hops=hops, ests=ests):
                    es2 = ests[w]
                    nc.tensor.matmul(sums[:], ones8[:], es2[:],
                                     start=(w == 0), stop=(w == NP - 1),
                                     perf_mode=DR)
                    for t in range(NT):
                        nc.tensor.matmul(hops[t][:],
                                         vot8[:, 2 * w:2 * w + 2, bass.ts(t, P)],
                                         es2[:],
                                         start=(w == 0), stop=(w == NP - 1),
                                         perf_mode=DR)
                    ests[w] = None

                for w in range(NP):
                    if w == 1 and pending_fin[0] is not None:
                        pending_fin[0]()
                        pending_fin[0] = None
                    es2 = est.tile([P, 2, IC], F8E5, tag="est",
                                   name=f"est{icn}_{w}")
                    for hh in range(2):
                        j = 2 * w + hh
                        sps = ps.tile([P, IC], F32, tag="mm", name=f"sps{icn}_{j}")
                        for u in range(2):
                            nc.tensor.matmul(sps[:],
                                             xt8[:, 2 * u:2 * u + 2, bass.ts(j, P)],
                                             qk8[:, 2 * u:2 * u + 2, bass.ts(icn, IC)],
                                             start=(u == 0), stop=(u == 1),
                                             perf_mode=DR)
                        nc.scalar.activation(es2[:, hh, :], sps[:],
                                             mybir.ActivationFunctionType.Exp,
                                             scale=1.0 / QK_SCALE)
                    ests[w] = es2
                    if w >= DEPTH:
                        consume(w - DEPTH)
                for w in range(NP - DEPTH, NP):
                    consume(w)
                pending_fin[0] = make_finalize(icn, sums, hops)
            pending_fin[0]()

    nc.compile()
    return nc


def _prep(inputs):
    s = float(C) ** -0.5
    wq = np.asarray(inputs["wq"], np.float64)
    wk = np.asarray(inputs["wk"], np.float64)
    wv = np.asarray(inputs["wv"], np.float64)
    wo = np.asarray(inputs["wo"], np.float64)
    bq = np.asarray(inputs["bq"], np.float64)
    bv = np.asarray(inputs["bv"], np.float64)
    bo = np.asarray(inputs["bo"], np.float64)
    gamma = np.asarray(inputs["gamma"], np.float64)
    beta = np.asarray(inputs["beta"], np.float64)
    Wqk = (wk.T @ wq).T * s      # lhsT layout [c_in, c_out]; Wqk.T is [c_out, c_in]
    Wov = (wo @ wv).T            # [c_in, c_out]
    bqkv = (wk.T @ bq) * s
    bovv = wo @ bv + bo
    # per-group partial matvecs with gamma folded: WgT[g, c] = sum_{c' in g} Wqk[c,c'] gamma[c']
    GS = C // G
    WgT = (Wqk * gamma[:, None]).reshape(G, GS, C).sum(axis=1)
    VgT = (Wov * gamma[:, None]).reshape(G, GS, C).sum(axis=1)
    consts = {
        "wqkT": np.ascontiguousarray(Wqk * QK_SCALE, np.float32),
        "wovT": np.ascontiguousarray(Wov * OV_SCALE).astype(ml_dtypes.bfloat16),
        "wgT": np.ascontiguousarray(WgT * QK_SCALE, np.float32),
        "vgT": np.ascontiguousarray(VgT, np.float32),
        "hqk": ((Wqk.T @ beta + bqkv) * QK_SCALE).astype(np.float32),
        "hov": (Wov.T @ beta + bovv).astype(np.float32),
        "gamma": np.asarray(inputs["gamma"], np.float32),
        "pg": np.ascontiguousarray(
            (np.arange(C)[:, None] // (C // G) == np.arange(G)[None, :])
            .astype(np.float32) / (C // G)),
        "sel": np.ascontiguousarray(
            (np.arange(G)[:, None] == np.arange(C)[None, :] // (C // G))
            .astype(np.float32)),
        "ones8": np.ones((P, 2 * P), ml_dtypes.float8_e4m3),
    }
    return consts


LAST_RESULTS = None


def kernel(**inputs) -> np.ndarray:
    global LAST_RESULTS
    if "nc" not in _CACHE:
        _CACHE["nc"] = _build()
    nc = _CACHE["nc"]
    consts = _prep(inputs)
    x = np.asarray(inputs["x"], np.float32)
    xb = x.reshape(B, C, L)
    in_maps = []
    for core in range(NCORES):
        b, chunk = divmod(core, 4)
        xr = np.roll(xb[b], -LQ * chunk, axis=1)
        xblk = np.ascontiguousarray(
            xr.reshape(NT, P, 2, L // 2).swapaxes(1, 2)).astype(ml_dtypes.bfloat16)
        xblk8 = xblk.astype(ml_dtypes.float8_e4m3)
        xf = np.ascontiguousarray(xr[:, :LQ].reshape(NT, P, LQ))
        in_maps.append({"x": xblk, "x8": xblk8, "xf": xf, **consts})
    res = bass_utils.run_bass_kernel_spmd(nc, in_maps, core_ids=list(range(NCORES)))
    LAST_RESULTS = res
    out = np.empty((B, C, L), np.float32)
    for core in range(NCORES):
        b, chunk = divmod(core, 4)
        out[b][:, LQ * chunk:LQ * (chunk + 1)] = res.results[core]["out"]
    return out.reshape(B, C, D, H, W)
